# revision 1
# baseline (speedup 1.0000x reference)
"""Trainium2 Bass kernel for nn_MhAttnBlock (GAT-style additive attention).

Reference computation (per batch b):
    Vproj = (V @ WV.T).reshape(k, H, 64)
    aK = K @ WK.T   (k, H)
    aQ = Q @ WQ.T   (q, H)
    w  = softmax_k(leaky_relu(aQ[q,h] + aK[k,h], 0.2))
    out[q, h*64+e] = sum_k w[q,k,h] * Vproj[k,h,e] + bias[h,e]

Key algebraic identity used on-device:
    exp(lrelu(s)) for s = aQ+aK equals max(A, B) = A + relu(B - A) with
       A = exp(aQ)*exp(aK)      (rank-1 in (q,k))
       B = exp(.2 aQ)*exp(.2 aK)
    So the score grid needs NO exp: PE builds D = B - A as a contraction-2
    matmul from tiny per-head exp vectors; a single relu pass (ACT + DVE
    split) doubles as the mandatory PSUM->SBUF move; the rank-1 A-term
    folds into the flash matmul as one extra accumulation row.  Softmax
    denominator = ones column appended to Vproj; bias folds in via
    Vproj += bias (numerator becomes num + bias*den, so num/den = out +
    bias exactly).

Sharding: data-parallel over batch B=8 across the 8 NeuronCores.
Matmuls run in float32r (single-pass PE multiply, fp32 accumulate).
"""

import sys

for _p in ("/opt/trn_rl_repo", "/root/.axon_site/_ro/trn_rl_repo"):
    if _p not in sys.path:
        sys.path.insert(0, _p)

import numpy as np

import concourse.bass as bass  # noqa: F401
import concourse.bacc as bacc
import concourse.mybir as mybir
import concourse.tile as tile
from concourse.bass_utils import run_bass_kernel_spmd
from concourse.masks import make_identity

F32 = mybir.dt.float32
F32R = mybir.dt.float32r
AF = mybir.ActivationFunctionType
ALU = mybir.AluOpType

B, QS, KS = 8, 1024, 1024
D = 512          # qdim = kdim = vdim
H, OD = 8, 64    # heads, head out dim
NEG = 0.2
NCORES = 8

KT = KS // 128   # 8 k-tiles
QT = QS // 128   # 8 q-tiles
DT = D // 128    # 4 d-tiles
QB = QS // 512   # 2 q-blocks of 512
HB = OD + 2      # 66: [out 64 | den ones | pad] (even for f32r matmul)
DEN = OD         # ones/den column index within a head block


def build_kernel():
    nc = bacc.Bacc()

    Qp = nc.declare_dram_parameter("Q", [QS, D], F32, isOutput=False)
    Kp = nc.declare_dram_parameter("K", [KS, D], F32, isOutput=False)
    Vp = nc.declare_dram_parameter("V", [KS, D], F32, isOutput=False)
    # WQext/WKext: (D, 2H), col 2h = W[h,:], col 2h+1 = 0.2*W[h,:]
    WQe = nc.declare_dram_parameter("WQext", [D, 2 * H], F32R, isOutput=False)
    WKe = nc.declare_dram_parameter("WKext", [D, 2 * H], F32R, isOutput=False)
    WVT = nc.declare_dram_parameter("WVT", [D, D], F32R, isOutput=False)
    # biasext: (1, H*HB): [bias[h,:64], 0, 0] per head
    BIA = nc.declare_dram_parameter("biasext", [1, H * HB], F32, isOutput=False)
    # sgn: (2H, 1): -1 on even partitions (negates eK1), +1 on odd
    SGN = nc.declare_dram_parameter("sgn", [2 * H, 1], F32, isOutput=False)
    OUT = nc.declare_dram_parameter("out", [QS, H * OD], F32, isOutput=True)

    with tile.TileContext(nc) as tc:
        with (
            tc.tile_pool(name="const", bufs=1) as constp,
            tc.tile_pool(name="big", bufs=1) as bigp,
            tc.tile_pool(name="stage", bufs=3) as stagep,
        ):
            # ---- constants ----
            ident = constp.tile([128, 128], F32, tag="ident")
            make_identity(nc, ident[:])
            sgn_sb = constp.tile([2 * H, 1], F32, tag="sgn")
            nc.sync.dma_start(sgn_sb[:], SGN[:])
            biasx = constp.tile([1, H * HB], F32, tag="biasx")
            nc.sync.dma_start(biasx[:], BIA[:])
            biasbc = constp.tile([128, H * HB], F32, tag="biasbc")
            ones1 = constp.tile([1, 128], F32, tag="ones1")
            nc.vector.memset(ones1[:], 1.0)
            with tc.tile_pool(name="psbb", bufs=1, space="PSUM") as psbbp:
                psbb = psbbp.tile([128, H * HB], F32, tag="psbb")
                nc.tensor.matmul(
                    psbb[:, 0:512], lhsT=ones1[:], rhs=biasx[:, 0:512],
                    start=True, stop=True,
                )
                nc.tensor.matmul(
                    psbb[:, 512:H * HB], lhsT=ones1[:],
                    rhs=biasx[:, 512:H * HB], start=True, stop=True,
                )
                nc.vector.tensor_copy(out=biasbc[:], in_=psbb[:])
            wq_sb = constp.tile([128, DT, 2 * H], F32R, tag="wq")
            nc.sync.dma_start(
                wq_sb[:], WQe.rearrange("(dt p) j -> p dt j", p=128)
            )
            wk_sb = constp.tile([128, DT, 2 * H], F32R, tag="wk")
            nc.sync.dma_start(
                wk_sb[:], WKe.rearrange("(dt p) j -> p dt j", p=128)
            )
            wv_sb = constp.tile([128, DT, D], F32R, tag="wv")
            nc.sync.dma_start(
                wv_sb[:], WVT.rearrange("(dt p) e -> p dt e", p=128)
            )

            xtcm = tc.tile_pool(name="xt", bufs=1)
            xtp = xtcm.__enter__()

            # ---- natural loads (chunked so transposes start early) ----
            natcm = tc.tile_pool(name="nat", bufs=1)
            natp = natcm.__enter__()

            def load_nat(dram, nt, tag):
                t_ = natp.tile([128, nt, D], F32, tag=tag)
                view = dram.rearrange("(p t) d -> p t d", p=128)
                for c in range(4):
                    lo, hi = c * nt // 4, (c + 1) * nt // 4
                    nc.sync.dma_start(t_[:, lo:hi], view[:, lo:hi])
                return t_

            q_nat = load_nat(Qp, QT, "qnat")
            k_nat = load_nat(Kp, KT, "knat")
            v_nat = load_nat(Vp, KT, "vnat")

            # ---- on-chip fp32 transposes via PE: X^T (d-part, x-free) ----
            with tc.tile_pool(name="pst", bufs=2, space="PSUM") as pstp:
                def transpose_in(nat, nt, tag):
                    xt = xtp.tile([128, DT, nt * 128], F32R, tag=tag)
                    for t in range(nt):
                        ps = pstp.tile([128, 512], F32, tag="pst")
                        for dt in range(DT):
                            nc.tensor.transpose(
                                ps[:, dt * 128:(dt + 1) * 128],
                                nat[:, t, dt * 128:(dt + 1) * 128],
                                ident[:],
                            )
                        # scatter 4 transposed chunks to their dt planes,
                        # alternating engines to balance ACT/DVE in setup
                        eng = nc.scalar.copy if t % 2 == 0 else (
                            nc.vector.tensor_copy
                        )
                        eng(
                            out=xt[:, :, t * 128:(t + 1) * 128],
                            in_=ps[:].rearrange("p (dt c) -> p dt c", dt=DT),
                        )
                    return xt

                qT = transpose_in(q_nat, QT, "qT")
                kT = transpose_in(k_nat, KT, "kT")
                vT = transpose_in(v_nat, KT, "vT")
            natcm.__exit__(None, None, None)

            # ---- projections ----
            with (
                tc.tile_pool(name="psproj", bufs=2, space="PSUM") as psprojp,
                tc.tile_pool(name="pspair", bufs=1, space="PSUM") as pspairp,
            ):
                # aQpair^T (2H, QS): row 2h = aQ_h, row 2h+1 = .2*aQ_h
                psq = pspairp.tile([2 * H, QS], F32, tag="pair")
                for half in range(QS // 512):
                    for dt in range(DT):
                        nc.tensor.matmul(
                            psq[:, half * 512:(half + 1) * 512],
                            lhsT=wq_sb[:, dt],
                            rhs=qT[:, dt, half * 512:(half + 1) * 512],
                            start=(dt == 0),
                            stop=(dt == DT - 1),
                        )
                eQ = bigp.tile([2 * H, QS], F32R, tag="eq")
                nc.scalar.activation(eQ[:], psq[:], AF.Exp)

                # aKpair^T: exp, negate even rows -> rows: -eK1, eK2
                psk = pspairp.tile([2 * H, KS], F32, tag="pair")
                for half in range(KS // 512):
                    for dt in range(DT):
                        nc.tensor.matmul(
                            psk[:, half * 512:(half + 1) * 512],
                            lhsT=wk_sb[:, dt],
                            rhs=kT[:, dt, half * 512:(half + 1) * 512],
                            start=(dt == 0),
                            stop=(dt == DT - 1),
                        )
                eK = bigp.tile([2 * H, KS], F32R, tag="ek")
                nc.scalar.activation(eK[:], psk[:], AF.Exp)
                nc.vector.tensor_scalar(
                    out=eK[:], in0=eK[:], scalar1=sgn_sb[:], scalar2=None,
                    op0=ALU.mult,
                )

                # aK natural (k-part, H) per k-tile -> eK1nat (128, KT*H)
                eK1n = bigp.tile([128, KT, H], F32R, tag="ek1n")
                for t in range(KT):
                    psn = psprojp.tile([128, H], F32, tag="psn")
                    for dt in range(DT):
                        nc.tensor.matmul(
                            psn[:],
                            lhsT=kT[:, dt, t * 128:(t + 1) * 128],
                            rhs=wk_sb[:, dt, 0:2 * H:2],
                            start=(dt == 0),
                            stop=(dt == DT - 1),
                        )
                    nc.scalar.activation(eK1n[:, t], psn[:], AF.Exp)

                # Vproj' (128, KT*(H*HB)): per head [Vproj_h + bias_h | 1 | 0]
                vp_sb = bigp.tile([128, KT, H * HB], F32R, tag="vp")
                for t in range(KT):
                    psv = psprojp.tile([128, 512], F32, tag="psv")
                    for dt in range(DT):
                        nc.tensor.matmul(
                            psv[:],
                            lhsT=vT[:, dt, t * 128:(t + 1) * 128],
                            rhs=wv_sb[:, dt],
                            start=(dt == 0),
                            stop=(dt == DT - 1),
                        )
                    nc.vector.tensor_tensor(
                        out=vp_sb[:, t].rearrange("p (h e) -> p h e", h=H)[
                            :, :, 0:OD
                        ],
                        in0=psv[:].rearrange("p (h e) -> p h e", h=H),
                        in1=biasbc[:].rearrange("p (h e) -> p h e", h=H)[
                            :, :, 0:OD
                        ],
                        op=ALU.add,
                    )
                # den ones column + zero pad column
                nc.vector.memset(
                    vp_sb[:].bitcast(F32).rearrange(
                        "p t (h e) -> p t h e", h=H
                    )[:, :, :, DEN:DEN + 1],
                    1.0,
                )
                nc.vector.memset(
                    vp_sb[:].bitcast(F32).rearrange(
                        "p t (h e) -> p t h e", h=H
                    )[:, :, :, DEN + 1:HB],
                    0.0,
                )

                # cV1'[block h] = sum_k eK1[k] * Vp'[k, block] (incl. den,pad)
                cv_sb = constp.tile([1, H * HB], F32R, tag="cv")
                for hh in range(2):  # two psum halves (heads 0-3, 4-7)
                    psc = psprojp.tile([1, 4 * HB], F32, tag="psc")
                    for hi in range(4):
                        h = hh * 4 + hi
                        for t in range(KT):
                            nc.tensor.matmul(
                                psc[:, hi * HB:(hi + 1) * HB],
                                lhsT=eK1n[:, t, h:h + 1],
                                rhs=vp_sb[:, t, h * HB:(h + 1) * HB],
                                start=(t == 0),
                                stop=(t == KT - 1),
                            )
                    nc.vector.tensor_copy(
                        out=cv_sb[:, hh * 4 * HB:(hh + 1) * 4 * HB], in_=psc[:]
                    )

            xtcm.__exit__(None, None, None)

            # ---- main grid: D = B - A, R = relu(D), flash matmul ----
            outF = bigp.tile([128, QT, H * OD], F32, tag="outf")
            with (
                tc.tile_pool(name="grid", bufs=1) as gridp,
                tc.tile_pool(name="psd", bufs=3, space="PSUM") as psdp,
                tc.tile_pool(name="pso", bufs=1, space="PSUM") as psop,
                tc.tile_pool(name="pst2", bufs=1, space="PSUM") as pst2p,
            ):
                ekh = []
                eqh = []
                for h in range(H):
                    # stage pair rows at base partition 0 (engine APs must
                    # start at partition 0/32/64/96; DMA may read anywhere)
                    ek_h = gridp.tile([2, KS], F32R, tag=f"ekh{h}")
                    nc.sync.dma_start(out=ek_h[:], in_=eK[2 * h:2 * h + 2, :])
                    ekh.append(ek_h)
                    eq_h = gridp.tile([2, QS], F32R, tag=f"eqh{h}")
                    nc.sync.dma_start(out=eq_h[:], in_=eQ[2 * h:2 * h + 2, :])
                    eqh.append(eq_h)
                outv = OUT.rearrange("(p t) e -> p t e", p=128)
                for qb in range(QB):
                    qs = qb * 512
                    for h in range(H):
                        ek_h, eq_h = ekh[h], eqh[h]
                        psO = psop.tile([HB, 512], F32, tag="pso")
                        for tp in range(KT // 2):  # k-tile pairs
                            psD = psdp.tile([128, 1024], F32, tag="psd")
                            for i in range(2):
                                t = tp * 2 + i
                                nc.tensor.matmul(
                                    psD[:, i * 512:(i + 1) * 512],
                                    lhsT=ek_h[:, t * 128:(t + 1) * 128],
                                    rhs=eq_h[:, qs:qs + 512],
                                    start=True,
                                    stop=True,
                                )
                            r_sb = stagep.tile([128, 1024], F32R, tag="r")
                            if tp == 1:  # give DVE one of the 4 relu pairs
                                nc.vector.tensor_scalar(
                                    out=r_sb[:], in0=psD[:], scalar1=0.0,
                                    scalar2=None, op0=ALU.max,
                                )
                            else:
                                nc.scalar.activation(r_sb[:], psD[:], AF.Relu)
                            for i in range(2):
                                t = tp * 2 + i
                                nc.tensor.matmul(
                                    psO[:],
                                    lhsT=vp_sb[:, t, h * HB:(h + 1) * HB],
                                    rhs=r_sb[:, i * 512:(i + 1) * 512],
                                    start=(t == 0),
                                    stop=False,
                                )
                        # rank-1 A-term: psO += cV1'_h (x) eQ1_h (pair row 0)
                        nc.tensor.matmul(
                            psO[:],
                            lhsT=cv_sb[0:1, h * HB:(h + 1) * HB],
                            rhs=eq_h[0:1, qs:qs + 512],
                            start=False,
                            stop=True,
                        )
                        # epilogue: copy out, transpose to q-major, divide
                        o_sb = stagep.tile([HB, 512], F32, tag="osb")
                        nc.vector.tensor_copy(out=o_sb[:], in_=psO[:])
                        ps2 = pst2p.tile([128, 4 * HB], F32, tag="ps2")
                        for c in range(4):
                            nc.tensor.transpose(
                                ps2[:, c * HB:(c + 1) * HB],
                                o_sb[:, c * 128:(c + 1) * 128],
                                ident[0:HB, 0:HB],
                            )
                        rden = stagep.tile([128, 4], F32, tag="rden")
                        nc.vector.reciprocal(
                            rden[:],
                            ps2[:].rearrange("p (c e) -> p c e", c=4)[
                                :, :, DEN:DEN + 1
                            ],
                        )
                        for c in range(4):
                            qt = qb * 4 + c
                            nc.vector.tensor_scalar(
                                out=outF[:, qt, h * OD:(h + 1) * OD],
                                in0=ps2[:, c * HB:c * HB + OD],
                                scalar1=rden[:, c:c + 1],
                                scalar2=None,
                                op0=ALU.mult,
                            )
                    # this qb's four q-tiles are complete: ship them now
                    nc.sync.dma_start(
                        out=outv[:, qb * 4:(qb + 1) * 4],
                        in_=outF[:, qb * 4:(qb + 1) * 4],
                    )
    nc.compile()
    return nc


_NC_CACHE = {}


def _get_nc():
    if "nc" not in _NC_CACHE:
        _NC_CACHE["nc"] = build_kernel()
    return _NC_CACHE["nc"]


def make_inmaps(Q, K, V, WQ, WK, WV, bias):
    Q = np.asarray(Q, np.float32)
    K = np.asarray(K, np.float32)
    V = np.asarray(V, np.float32)
    WQ = np.asarray(WQ, np.float32)
    WK = np.asarray(WK, np.float32)
    WV = np.asarray(WV, np.float32)
    bias = np.asarray(bias, np.float32)

    def ext(W):  # (H, D) -> (D, 2H), col 2h = W[h], col 2h+1 = .2*W[h]
        e = np.empty((D, 2 * H), np.float32)
        e[:, 0::2] = W.T
        e[:, 1::2] = NEG * W.T
        return e

    wqe = ext(WQ)
    wke = ext(WK)
    wvt = np.ascontiguousarray(WV.T)
    biasext = np.zeros((1, H * HB), np.float32)
    biasext.reshape(H, HB)[:, 0:OD] = bias
    sgn = np.tile(np.array([[-1.0], [1.0]], np.float32), (H, 1))

    in_maps = []
    for b in range(NCORES):
        in_maps.append({
            "Q": np.ascontiguousarray(Q[b]),
            "K": np.ascontiguousarray(K[b]),
            "V": np.ascontiguousarray(V[b]),
            "WQext": wqe,
            "WKext": wke,
            "WVT": wvt,
            "biasext": biasext,
            "sgn": sgn,
        })
    return in_maps


def kernel(Q, K, V, WQ, WK, WV, bias):
    nc = _get_nc()
    in_maps = make_inmaps(Q, K, V, WQ, WK, WV, bias)
    res = run_bass_kernel_spmd(nc, in_maps, list(range(NCORES)))
    out = np.stack([res.results[b]["out"] for b in range(NCORES)], axis=0)
    return out



# revision 8
# speedup vs baseline: 1.2309x; 1.2309x over previous
"""Trainium2 Bass kernel for nn_MhAttnBlock (GAT-style additive attention).

Reference computation (per batch b):
    Vproj = (V @ WV.T).reshape(k, H, 64)
    aK = K @ WK.T   (k, H)
    aQ = Q @ WQ.T   (q, H)
    w  = softmax_k(leaky_relu(aQ[q,h] + aK[k,h], 0.2))
    out[q, h*64+e] = sum_k w[q,k,h] * Vproj[k,h,e] + bias[h,e]

Key algebraic identity used on-device:
    exp(lrelu(s)) for s = aQ+aK equals max(A, B) = A + relu(B - A) with
       A = exp(aQ)*exp(aK)      (rank-1 in (q,k))
       B = exp(.2 aQ)*exp(.2 aK)
    So the score grid needs NO exp: PE builds D = B - A as a contraction-2
    matmul from tiny per-head exp vectors; a relu pass (split across ACT
    and DVE) doubles as the mandatory PSUM->SBUF move; the rank-1 A-term
    folds into the flash matmul as a C=1 accumulation.  Softmax
    denominator = ones column appended to Vproj; bias folds in via
    Vproj += bias (numerator becomes num + bias*den, so num/den = out +
    bias exactly).

This version (v2):
  - All heavy matmuls in bf16 (fp32r's LOW_HIGH replicated mode runs the
    PE at ~half rate / trips the chip power throttle when 8 cores run).
  - Flash matmul flipped: lhsT = relu-grid chunk [128k, 128q], rhs =
    Vproj head block [128k, 66] -> psO [128q, 66].  Output lands q-major
    so the entire PE-transpose epilogue of v1 disappears.
  - Front phase interleaved per-DMA-chunk (K, V, Q load order) so
    transposes/projections hide under the HBM loads.
  - Grid software-pipelined: score matmuls for tile-pair tp+1 issue
    before flash matmuls of tp, so the PE never stalls on the relu.

Sharding: data-parallel over batch B=8 across the 8 NeuronCores.
"""

import sys

for _p in ("/opt/trn_rl_repo", "/root/.axon_site/_ro/trn_rl_repo"):
    if _p not in sys.path:
        sys.path.insert(0, _p)

import numpy as np
import ml_dtypes

import concourse.bass as bass  # noqa: F401
import concourse.bacc as bacc
import concourse.mybir as mybir
import concourse.tile as tile
from concourse.bass_utils import run_bass_kernel_spmd
from concourse.masks import make_identity

F32 = mybir.dt.float32
BF16 = mybir.dt.bfloat16
AF = mybir.ActivationFunctionType
ALU = mybir.AluOpType

B, QS, KS = 8, 1024, 1024
D = 512          # qdim = kdim = vdim
H, OD = 8, 64    # heads, head out dim
NEG = 0.2
NCORES = 8

KT = KS // 128   # 8 k-tiles
QT = QS // 128   # 8 q-tiles
DT = D // 128    # 4 d-tiles
QB = QS // 512   # 2 q-blocks of 512
NCH = 4          # dma chunks per input tensor (2 k/q-tiles each)
HB = OD + 2      # 66: [out 64 | den ones | pad]
DEN = OD         # ones/den column index within a head block


def build_kernel():
    nc = bacc.Bacc()

    Qp = nc.declare_dram_parameter("Q", [QS, D], F32, isOutput=False)
    Kp = nc.declare_dram_parameter("K", [KS, D], F32, isOutput=False)
    Vp = nc.declare_dram_parameter("V", [KS, D], F32, isOutput=False)
    # WQext/WKext: (D, 2H), col 2h = W[h,:], col 2h+1 = 0.2*W[h,:]
    WQe = nc.declare_dram_parameter("WQext", [D, 2 * H], BF16, isOutput=False)
    WKe = nc.declare_dram_parameter("WKext", [D, 2 * H], BF16, isOutput=False)
    WVT = nc.declare_dram_parameter("WVT", [D, D], BF16, isOutput=False)
    # biasext: (1, H*HB): [bias[h,:64], 0, 0] per head
    BIA = nc.declare_dram_parameter("biasext", [1, H * HB], BF16, isOutput=False)
    # sgn: (2H, 1): -1 on even partitions (negates eK1), +1 on odd
    SGN = nc.declare_dram_parameter("sgn", [2 * H, 1], F32, isOutput=False)
    OUT = nc.declare_dram_parameter("out", [QS, H * OD], F32, isOutput=True)

    with tile.TileContext(nc) as tc:
        with (
            tc.tile_pool(name="const", bufs=1) as constp,
            tc.tile_pool(name="big", bufs=1) as bigp,
            tc.tile_pool(name="stage", bufs=3) as stagep,
        ):
            # ---- constants ----
            ident = constp.tile([128, 128], F32, tag="ident")
            make_identity(nc, ident[:])
            sgn_sb = constp.tile([2 * H, 1], F32, tag="sgn")
            nc.sync.dma_start(sgn_sb[:], SGN[:])
            biasx = constp.tile([1, H * HB], BF16, tag="biasx")
            nc.sync.dma_start(biasx[:], BIA[:])
            biasbc = constp.tile([128, H * HB], F32, tag="biasbc")
            ones1 = constp.tile([1, 128], BF16, tag="ones1")
            nc.vector.memset(ones1[:], 1.0)
            # weights (loaded early; small)
            wq_sb = constp.tile([128, DT, 2 * H], BF16, tag="wq")
            nc.sync.dma_start(
                wq_sb[:], WQe.rearrange("(dt p) j -> p dt j", p=128)
            )
            wk_sb = constp.tile([128, DT, 2 * H], BF16, tag="wk")
            nc.sync.dma_start(
                wk_sb[:], WKe.rearrange("(dt p) j -> p dt j", p=128)
            )
            wv_sb = constp.tile([128, DT, D], BF16, tag="wv")
            nc.sync.dma_start(
                wv_sb[:], WVT.rearrange("(dt p) e -> p dt e", p=128)
            )

            # ---- input loads (K, V, Q order; 4 chunks each) ----
            natcm = tc.tile_pool(name="nat", bufs=1)
            natp = natcm.__enter__()

            def load_nat(dram, nt, tag):
                t_ = natp.tile([128, nt, D], F32, tag=tag)
                view = dram.rearrange("(p t) d -> p t d", p=128)
                for c in range(NCH):
                    lo, hi = c * nt // NCH, (c + 1) * nt // NCH
                    nc.sync.dma_start(t_[:, lo:hi], view[:, lo:hi])
                return t_

            k_nat = load_nat(Kp, KT, "knat")
            v_nat = load_nat(Vp, KT, "vnat")
            q_nat = load_nat(Qp, QT, "qnat")

            # bias broadcast to 128 partitions via tiny bf16 matmul
            with tc.tile_pool(name="psbb", bufs=1, space="PSUM") as psbbp:
                psbb = psbbp.tile([128, H * HB], F32, tag="psbb")
                nc.tensor.matmul(
                    psbb[:, 0:512], lhsT=ones1[:], rhs=biasx[:, 0:512],
                    start=True, stop=True,
                )
                nc.tensor.matmul(
                    psbb[:, 512:H * HB], lhsT=ones1[:],
                    rhs=biasx[:, 512:H * HB], start=True, stop=True,
                )
                nc.vector.tensor_copy(out=biasbc[:], in_=psbb[:])

            # ---- transposed (d-part) bf16 copies + projections,
            #      interleaved per chunk ----
            xtcm = tc.tile_pool(name="xt", bufs=1)
            xtp = xtcm.__enter__()
            kT = xtp.tile([128, DT, KS], BF16, tag="kT")
            vT = xtp.tile([128, DT, KS], BF16, tag="vT")
            qT = xtp.tile([128, DT, QS], BF16, tag="qT")

            with (
                tc.tile_pool(name="pst", bufs=2, space="PSUM") as pstp,
                tc.tile_pool(name="pspair", bufs=1, space="PSUM") as pspairp,
                tc.tile_pool(name="psproj", bufs=2, space="PSUM") as psprojp,
            ):
                def transpose_chunk(nat, xt, c):
                    # chunk c holds seq-tiles 2c, 2c+1
                    for t in (2 * c, 2 * c + 1):
                        ps = pstp.tile([128, 512], F32, tag="pst")
                        for dt in range(DT):
                            nc.tensor.transpose(
                                ps[:, dt * 128:(dt + 1) * 128],
                                nat[:, t, dt * 128:(dt + 1) * 128],
                                ident[:],
                            )
                        eng = nc.scalar.copy if t % 2 == 0 else (
                            nc.vector.tensor_copy
                        )
                        eng(
                            out=xt[:, :, t * 128:(t + 1) * 128],
                            in_=ps[:].rearrange("p (dt c) -> p dt c", dt=DT),
                        )

                # pair-projection psum tiles [2H, seq] fp32 (2 banks each)
                psk = pspairp.tile([2 * H, KS], F32, tag="pair")
                eK = bigp.tile([2 * H, KS], BF16, tag="ek")
                ekf = bigp.tile([2 * H, KS], F32, tag="ekf")
                eK1n = bigp.tile([128, KT, H], BF16, tag="ek1n")

                def pair_proj(ps, xt, half, w_sb):
                    for dt in range(DT):
                        nc.tensor.matmul(
                            ps[:, half * 512:(half + 1) * 512],
                            lhsT=w_sb[:, dt],
                            rhs=xt[:, dt, half * 512:(half + 1) * 512],
                            start=(dt == 0),
                            stop=(dt == DT - 1),
                        )

                def ek_finish(half):
                    sl = slice(half * 512, (half + 1) * 512)
                    nc.scalar.activation(ekf[:, sl], psk[:, sl], AF.Exp)
                    nc.vector.tensor_scalar(
                        out=eK[:, sl], in0=ekf[:, sl], scalar1=sgn_sb[:],
                        scalar2=None, op0=ALU.mult,
                    )

                def ek1n_tile(t):
                    psn_full = psprojp.tile([128, 512], F32, tag="proj",
                                            name=f"psn{t}")
                    psn = psn_full[:, 0:H]
                    for dt in range(DT):
                        nc.tensor.matmul(
                            psn,
                            lhsT=kT[:, dt, t * 128:(t + 1) * 128],
                            rhs=wk_sb[:, dt, 0:2 * H:2],
                            start=(dt == 0),
                            stop=(dt == DT - 1),
                        )
                    nc.scalar.activation(eK1n[:, t], psn, AF.Exp)

                # --- K path ---
                transpose_chunk(k_nat, kT, 0)
                transpose_chunk(k_nat, kT, 1)
                pair_proj(psk, kT, 0, wk_sb)
                ek_finish(0)
                for t in range(0, 4):
                    ek1n_tile(t)
                transpose_chunk(k_nat, kT, 2)
                transpose_chunk(k_nat, kT, 3)
                pair_proj(psk, kT, 1, wk_sb)
                ek_finish(1)
                for t in range(4, 8):
                    ek1n_tile(t)

                # --- V path: Vproj' per tile: [Vproj_h + bias_h | 1 | 0] ---
                vp_sb = bigp.tile([128, KT, H * HB], BF16, tag="vp")
                nc.vector.memset(
                    vp_sb[:].rearrange("p t (h e) -> p t h e", h=H)[
                        :, :, :, DEN:DEN + 1
                    ],
                    1.0,
                )
                nc.vector.memset(
                    vp_sb[:].rearrange("p t (h e) -> p t h e", h=H)[
                        :, :, :, DEN + 1:HB
                    ],
                    0.0,
                )

                def vp_tile(t):
                    psv = psprojp.tile([128, 512], F32, tag="proj", name=f"psv{t}")
                    for dt in range(DT):
                        nc.tensor.matmul(
                            psv[:],
                            lhsT=vT[:, dt, t * 128:(t + 1) * 128],
                            rhs=wv_sb[:, dt],
                            start=(dt == 0),
                            stop=(dt == DT - 1),
                        )
                    nc.vector.tensor_tensor(
                        out=vp_sb[:, t].rearrange("p (h e) -> p h e", h=H)[
                            :, :, 0:OD
                        ],
                        in0=psv[:].rearrange("p (h e) -> p h e", h=H),
                        in1=biasbc[:].rearrange("p (h e) -> p h e", h=H)[
                            :, :, 0:OD
                        ],
                        op=ALU.add,
                    )

                for c in range(NCH):
                    transpose_chunk(v_nat, vT, c)
                    vp_tile(2 * c)
                    vp_tile(2 * c + 1)

                # --- cv: cv1'[h] = sum_k exp(aK_h)[k] * Vp'[k, block h] ---
                # one [8, 264] matmul pair per k-tile; head h's block sits at
                # cols h*66 of the concatenated [8, 528] result (diag blocks)
                with tc.tile_pool(name="cvp", bufs=1, space="PSUM") as cvpp:
                    # [H, 2, 512] so each half's [8, 264] matmul output sits
                    # at a PSUM bank boundary (offsets 0 and 2048 bytes)
                    cvps = cvpp.tile([H, 2, 512], F32, tag="cvps")
                    for t in range(KT):
                        for hh in range(2):
                            nc.tensor.matmul(
                                cvps[:, hh, 0:4 * HB],
                                lhsT=eK1n[:, t],
                                rhs=vp_sb[:, t, hh * 4 * HB:(hh + 1) * 4 * HB],
                                start=(t == 0),
                                stop=(t == KT - 1),
                            )
                    cvf = bigp.tile([H, 2 * 4 * HB], BF16, tag="cvf")
                    for hh in range(2):
                        nc.vector.tensor_copy(
                            out=cvf[:, hh * 4 * HB:(hh + 1) * 4 * HB],
                            in_=cvps[:, hh, 0:4 * HB],
                        )
                # gather diagonal blocks to partition 0: cv_sb[0, h*66+j]
                cv_sb = constp.tile([1, H * HB], BF16, tag="cv")
                for h in range(H):
                    nc.sync.dma_start(
                        out=cv_sb[:, h * HB:(h + 1) * HB],
                        in_=cvf[h:h + 1, h * HB:(h + 1) * HB],
                    )

                # --- Q path ---
                psq = pspairp.tile([2 * H, QS], F32, tag="pair")
                eQ = bigp.tile([2 * H, QS], BF16, tag="eq")
                transpose_chunk(q_nat, qT, 0)
                transpose_chunk(q_nat, qT, 1)
                pair_proj(psq, qT, 0, wq_sb)
                nc.scalar.activation(eQ[:, 0:512], psq[:, 0:512], AF.Exp)
                transpose_chunk(q_nat, qT, 2)
                transpose_chunk(q_nat, qT, 3)
                pair_proj(psq, qT, 1, wq_sb)
                nc.scalar.activation(eQ[:, 512:QS], psq[:, 512:QS], AF.Exp)

            xtcm.__exit__(None, None, None)
            natcm.__exit__(None, None, None)

            # ---- stage pair rows at partition 0/1 (engine APs must start
            #      at partition 0/32/64/96; DMA may read anywhere).  Issued
            #      on the idle gpsimd DGE queue to keep sync free for loads.
            eks = bigp.tile([2, H, KS], BF16, tag="eks")
            eqs = bigp.tile([2, H, QS], BF16, tag="eqs")
            for h in range(H):
                nc.gpsimd.dma_start(
                    out=eks[:, h], in_=eK[2 * h:2 * h + 2, :]
                )
                nc.gpsimd.dma_start(
                    out=eqs[:, h], in_=eQ[2 * h:2 * h + 2, :]
                )

            # ---- main grid: score D = B - A, relu, flipped flash ----
            # PSUM accumulation chains within one bank must be strictly
            # sequential (no two open groups in a bank region), so each
            # (qb,h) runs its 4 q-chunk chains back to back; the software
            # pipeline instead runs one full (qb,h) iteration ahead on the
            # score side.
            outv = OUT.rearrange("(p t) e -> p t e", p=128)
            with (
                tc.tile_pool(name="psd", bufs=3, space="PSUM") as psdp,
                tc.tile_pool(name="pso", bufs=2, space="PSUM") as psop,
                tc.tile_pool(name="rpool", bufs=9) as rp,
                tc.tile_pool(name="outf", bufs=4) as outfp,
            ):
                NIT = QB * H  # 16 iterations, j -> (qb, h)
                TP = KT // 2
                psO = [None] * NIT
                rsb = [None] * NIT

                def emit_scores(j):
                    qb, h = divmod(j, H)
                    qs = qb * 512
                    rsb[j] = []
                    for tp in range(TP):
                        ps = psdp.tile([128, 1024], F32, tag="psd",
                                       name=f"psD{j}_{tp}")
                        for i in range(2):
                            t = tp * 2 + i
                            nc.tensor.matmul(
                                ps[:, i * 512:(i + 1) * 512],
                                lhsT=eks[:, h, t * 128:(t + 1) * 128],
                                rhs=eqs[:, h, qs:qs + 512],
                                start=True, stop=True,
                            )
                        r = rp.tile([128, 1024], BF16, tag="r",
                                    name=f"r{j}_{tp}")
                        rsb[j].append(r)
                        # relu split: ACT 576 cols, DVE 448 cols
                        nc.scalar.activation(r[:, 0:576], ps[:, 0:576],
                                             AF.Relu)
                        nc.vector.tensor_scalar(
                            out=r[:, 576:1024], in0=ps[:, 576:1024],
                            scalar1=0.0, scalar2=None, op0=ALU.max,
                        )

                def emit_flashepi(j):
                    qb, h = divmod(j, H)
                    qs = qb * 512
                    pso_t = psop.tile([128, 4 * HB], F32, tag="pso",
                                      name=f"psO{j}")
                    psO[j] = pso_t
                    for c in range(4):
                        # rank-1 A-term opens chunk c's accumulation chain
                        nc.tensor.matmul(
                            pso_t[:, c * HB:(c + 1) * HB],
                            lhsT=eqs[0:1, h, qs + c * 128:qs + (c + 1) * 128],
                            rhs=cv_sb[0:1, h * HB:(h + 1) * HB],
                            start=True, stop=False,
                        )
                        for tp in range(TP):
                            r = rsb[j][tp]
                            for i in range(2):
                                t = tp * 2 + i
                                nc.tensor.matmul(
                                    pso_t[:, c * HB:(c + 1) * HB],
                                    lhsT=r[:, i * 512 + c * 128:
                                           i * 512 + (c + 1) * 128],
                                    rhs=vp_sb[:, t, h * HB:(h + 1) * HB],
                                    start=False, stop=(t == KT - 1),
                                )
                    rsb[j] = None
                    # epilogue: reciprocal of den column, scale, store
                    rden = stagep.tile([128, 4], F32, tag="rden",
                                       name=f"rden{j}")
                    nc.vector.reciprocal(
                        rden[:],
                        pso_t[:].rearrange("p (c e) -> p c e", c=4)[
                            :, :, DEN:DEN + 1
                        ],
                    )
                    oF = outfp.tile([128, 4 * OD], F32, tag="of",
                                    name=f"oF{j}")
                    for c in range(4):
                        nc.vector.tensor_scalar(
                            out=oF[:, c * OD:(c + 1) * OD],
                            in0=pso_t[:, c * HB:c * HB + OD],
                            scalar1=rden[:, c:c + 1],
                            scalar2=None,
                            op0=ALU.mult,
                        )
                    nc.sync.dma_start(
                        out=outv[:, qb * 4:(qb + 1) * 4, h * OD:(h + 1) * OD],
                        in_=oF[:].rearrange("p (c e) -> p c e", c=4),
                    )
                    psO[j] = None

                emit_scores(0)
                for j in range(NIT):
                    if j + 1 < NIT:
                        emit_scores(j + 1)
                    emit_flashepi(j)
    nc.compile()
    return nc


_NC_CACHE = {}


def _get_nc():
    if "nc" not in _NC_CACHE:
        _NC_CACHE["nc"] = build_kernel()
    return _NC_CACHE["nc"]


def make_inmaps(Q, K, V, WQ, WK, WV, bias):
    Q = np.asarray(Q, np.float32)
    K = np.asarray(K, np.float32)
    V = np.asarray(V, np.float32)
    WQ = np.asarray(WQ, np.float32)
    WK = np.asarray(WK, np.float32)
    WV = np.asarray(WV, np.float32)
    bias = np.asarray(bias, np.float32)

    def ext(W):  # (H, D) -> (D, 2H), col 2h = W[h], col 2h+1 = .2*W[h]
        e = np.empty((D, 2 * H), np.float32)
        e[:, 0::2] = W.T
        e[:, 1::2] = NEG * W.T
        return e.astype(ml_dtypes.bfloat16)

    wqe = ext(WQ)
    wke = ext(WK)
    wvt = np.ascontiguousarray(WV.T).astype(ml_dtypes.bfloat16)
    biasext = np.zeros((1, H * HB), np.float32)
    biasext.reshape(H, HB)[:, 0:OD] = bias
    biasext = biasext.astype(ml_dtypes.bfloat16)
    sgn = np.tile(np.array([[-1.0], [1.0]], np.float32), (H, 1))

    in_maps = []
    for b in range(NCORES):
        in_maps.append({
            "Q": np.ascontiguousarray(Q[b]),
            "K": np.ascontiguousarray(K[b]),
            "V": np.ascontiguousarray(V[b]),
            "WQext": wqe,
            "WKext": wke,
            "WVT": wvt,
            "biasext": biasext,
            "sgn": sgn,
        })
    return in_maps


def kernel(Q, K, V, WQ, WK, WV, bias):
    nc = _get_nc()
    in_maps = make_inmaps(Q, K, V, WQ, WK, WV, bias)
    res = run_bass_kernel_spmd(nc, in_maps, list(range(NCORES)))
    out = np.stack([res.results[b]["out"] for b in range(NCORES)], axis=0)
    return out


# revision 10
# speedup vs baseline: 1.3436x; 1.0916x over previous
"""Trainium2 Bass kernel for nn_MhAttnBlock (GAT-style additive attention).

Reference computation (per batch b):
    Vproj = (V @ WV.T).reshape(k, H, 64)
    aK = K @ WK.T   (k, H)
    aQ = Q @ WQ.T   (q, H)
    w  = softmax_k(leaky_relu(aQ[q,h] + aK[k,h], 0.2))
    out[q, h*64+e] = sum_k w[q,k,h] * Vproj[k,h,e] + bias[h,e]

Key algebraic identity used on-device:
    exp(lrelu(s)) for s = aQ+aK equals max(A, B) = A + relu(B - A) with
       A = exp(aQ)*exp(aK)      (rank-1 in (q,k))
       B = exp(.2 aQ)*exp(.2 aK)
    So the score grid needs NO exp: PE builds D = B - A as a contraction-2
    matmul from tiny per-head exp vectors; a relu pass (split across ACT
    and DVE) doubles as the mandatory PSUM->SBUF move; the rank-1 A-term
    folds into the flash matmul as a C=1 accumulation.  Softmax
    denominator = ones column appended to Vproj; bias folds in via
    Vproj += bias (numerator becomes num + bias*den, so num/den = out +
    bias exactly).

This version (v2):
  - All heavy matmuls in bf16 (fp32r's LOW_HIGH replicated mode runs the
    PE at ~half rate / trips the chip power throttle when 8 cores run).
  - Flash matmul flipped: lhsT = relu-grid chunk [128k, 128q], rhs =
    Vproj head block [128k, 66] -> psO [128q, 66].  Output lands q-major
    so the entire PE-transpose epilogue of v1 disappears.
  - Front phase interleaved per-DMA-chunk (K, V, Q load order) so
    transposes/projections hide under the HBM loads.
  - Grid software-pipelined: score matmuls for tile-pair tp+1 issue
    before flash matmuls of tp, so the PE never stalls on the relu.

Sharding: data-parallel over batch B=8 across the 8 NeuronCores.
"""

import sys

for _p in ("/opt/trn_rl_repo", "/root/.axon_site/_ro/trn_rl_repo"):
    if _p not in sys.path:
        sys.path.insert(0, _p)

import numpy as np
import ml_dtypes

import concourse.bass as bass  # noqa: F401
import concourse.bacc as bacc
import concourse.mybir as mybir
import concourse.tile as tile
from concourse.bass_utils import run_bass_kernel_spmd

F32 = mybir.dt.float32
BF16 = mybir.dt.bfloat16
AF = mybir.ActivationFunctionType
ALU = mybir.AluOpType

B, QS, KS = 8, 1024, 1024
D = 512          # qdim = kdim = vdim
H, OD = 8, 64    # heads, head out dim
NEG = 0.2
NCORES = 8

KT = KS // 128   # 8 k-tiles
QT = QS // 128   # 8 q-tiles
DT = D // 128    # 4 d-tiles
QB = QS // 512   # 2 q-blocks of 512
NCH = 4          # dma chunks per input tensor (2 k/q-tiles each)
HB = OD + 2      # 66: [out 64 | den ones | pad]
DEN = OD         # ones/den column index within a head block


def build_kernel():
    nc = bacc.Bacc()

    # host passes Q/K/V pre-transposed (d-major) and bf16-cast; the PE
    # transposes + fp32 loads of earlier versions disappear entirely
    QTd = nc.declare_dram_parameter("QT", [D, QS], BF16, isOutput=False)
    KTd = nc.declare_dram_parameter("KT", [D, KS], BF16, isOutput=False)
    VTd = nc.declare_dram_parameter("VT", [D, KS], BF16, isOutput=False)
    # WQext/WKext: (D, 2H), col 2h = W[h,:], col 2h+1 = 0.2*W[h,:]
    WQe = nc.declare_dram_parameter("WQext", [D, 2 * H], BF16, isOutput=False)
    WKe = nc.declare_dram_parameter("WKext", [D, 2 * H], BF16, isOutput=False)
    WVT = nc.declare_dram_parameter("WVT", [D, D], BF16, isOutput=False)
    # biasext: (1, H*HB): [bias[h,:64], 0, 0] per head
    BIA = nc.declare_dram_parameter("biasext", [1, H * HB], BF16, isOutput=False)
    # sgn: (2H, 1): -1 on even partitions (negates eK1), +1 on odd
    SGN = nc.declare_dram_parameter("sgn", [2 * H, 1], F32, isOutput=False)
    OUT = nc.declare_dram_parameter("out", [QS, H * OD], F32, isOutput=True)

    with tile.TileContext(nc) as tc:
        with (
            tc.tile_pool(name="const", bufs=1) as constp,
            tc.tile_pool(name="big", bufs=1) as bigp,
            tc.tile_pool(name="stage", bufs=3) as stagep,
        ):
            # ---- tiny constants on the gpsimd DGE queue (sync stays free
            #      for the input loads; scalar queue takes the weights) ----
            sgn_sb = constp.tile([2 * H, 1], F32, tag="sgn")
            nc.gpsimd.dma_start(sgn_sb[:], SGN[:])
            biasx = constp.tile([1, H * HB], BF16, tag="biasx")
            nc.gpsimd.dma_start(biasx[:], BIA[:])
            biasbc = constp.tile([128, H * HB], F32, tag="biasbc")
            ones1 = constp.tile([1, 128], BF16, tag="ones1")
            nc.vector.memset(ones1[:], 1.0)
            wk_sb = constp.tile([128, DT, 2 * H], BF16, tag="wk")
            nc.scalar.dma_start(
                wk_sb[:], WKe.rearrange("(dt p) j -> p dt j", p=128)
            )
            wq_sb = constp.tile([128, DT, 2 * H], BF16, tag="wq")
            nc.scalar.dma_start(
                wq_sb[:], WQe.rearrange("(dt p) j -> p dt j", p=128)
            )
            wv_sb = constp.tile([128, DT, D], BF16, tag="wv")
            nc.scalar.dma_start(
                wv_sb[:], WVT.rearrange("(dt p) e -> p dt e", p=128)
            )

            # ---- transposed input loads (K, V, Q order; 4 seq-chunks
            #      each so projections can start as data arrives) ----
            xtcm = tc.tile_pool(name="xt", bufs=1)
            xtp = xtcm.__enter__()

            def load_xt(dram, ns, tag):
                t_ = xtp.tile([128, DT, ns], BF16, tag=tag)
                view = dram.rearrange("(dt p) s -> p dt s", p=128)
                for c in range(NCH):
                    lo, hi = c * ns // NCH, (c + 1) * ns // NCH
                    nc.sync.dma_start(t_[:, :, lo:hi], view[:, :, lo:hi])
                return t_

            kT = load_xt(KTd, KS, "kT")
            vT = load_xt(VTd, KS, "vT")
            qT = load_xt(QTd, QS, "qT")

            # bias broadcast to 128 partitions via tiny bf16 matmul
            with tc.tile_pool(name="psbb", bufs=1, space="PSUM") as psbbp:
                psbb = psbbp.tile([128, H * HB], F32, tag="psbb")
                nc.tensor.matmul(
                    psbb[:, 0:512], lhsT=ones1[:], rhs=biasx[:, 0:512],
                    start=True, stop=True,
                )
                nc.tensor.matmul(
                    psbb[:, 512:H * HB], lhsT=ones1[:],
                    rhs=biasx[:, 512:H * HB], start=True, stop=True,
                )
                nc.vector.tensor_copy(out=biasbc[:], in_=psbb[:])

            with (
                tc.tile_pool(name="pspair", bufs=1, space="PSUM") as pspairp,
                tc.tile_pool(name="psproj", bufs=2, space="PSUM") as psprojp,
            ):
                # pair-projection psum tiles [2H, seq] fp32 (2 banks each)
                psk = pspairp.tile([2 * H, KS], F32, tag="pair")
                eK = bigp.tile([2 * H, KS], BF16, tag="ek")
                ekf = bigp.tile([2 * H, KS], F32, tag="ekf")
                eK1n = bigp.tile([128, KT, H], BF16, tag="ek1n")

                def pair_proj(ps, xt, half, w_sb):
                    for dt in range(DT):
                        nc.tensor.matmul(
                            ps[:, half * 512:(half + 1) * 512],
                            lhsT=w_sb[:, dt],
                            rhs=xt[:, dt, half * 512:(half + 1) * 512],
                            start=(dt == 0),
                            stop=(dt == DT - 1),
                        )

                def ek_finish(half):
                    sl = slice(half * 512, (half + 1) * 512)
                    nc.scalar.activation(ekf[:, sl], psk[:, sl], AF.Exp)
                    nc.vector.tensor_scalar(
                        out=eK[:, sl], in0=ekf[:, sl], scalar1=sgn_sb[:],
                        scalar2=None, op0=ALU.mult,
                    )

                def ek1n_tile(t):
                    psn_full = psprojp.tile([128, 512], F32, tag="proj",
                                            name=f"psn{t}")
                    psn = psn_full[:, 0:H]
                    for dt in range(DT):
                        nc.tensor.matmul(
                            psn,
                            lhsT=kT[:, dt, t * 128:(t + 1) * 128],
                            rhs=wk_sb[:, dt, 0:2 * H:2],
                            start=(dt == 0),
                            stop=(dt == DT - 1),
                        )
                    nc.scalar.activation(eK1n[:, t], psn, AF.Exp)

                # --- K path ---
                pair_proj(psk, kT, 0, wk_sb)
                ek_finish(0)
                for t in range(0, 4):
                    ek1n_tile(t)
                pair_proj(psk, kT, 1, wk_sb)
                ek_finish(1)
                for t in range(4, 8):
                    ek1n_tile(t)

                # stage eK pair rows at partitions 0/1 (engine APs must
                # start at partition 0/32/64/96; DMA may read anywhere)
                eks = bigp.tile([2, H, KS], BF16, tag="eks")
                for h in range(H):
                    nc.gpsimd.dma_start(
                        out=eks[:, h], in_=eK[2 * h:2 * h + 2, :]
                    )

                # --- V path: Vproj' per tile: [Vproj_h + bias_h | 1 | 0] ---
                vp_sb = bigp.tile([128, KT, H * HB], BF16, tag="vp")
                nc.vector.memset(
                    vp_sb[:].rearrange("p t (h e) -> p t h e", h=H)[
                        :, :, :, DEN:DEN + 1
                    ],
                    1.0,
                )
                nc.vector.memset(
                    vp_sb[:].rearrange("p t (h e) -> p t h e", h=H)[
                        :, :, :, DEN + 1:HB
                    ],
                    0.0,
                )

                def vp_tile(t):
                    psv = psprojp.tile([128, 512], F32, tag="proj",
                                       name=f"psv{t}")
                    for dt in range(DT):
                        nc.tensor.matmul(
                            psv[:],
                            lhsT=vT[:, dt, t * 128:(t + 1) * 128],
                            rhs=wv_sb[:, dt],
                            start=(dt == 0),
                            stop=(dt == DT - 1),
                        )
                    nc.vector.tensor_tensor(
                        out=vp_sb[:, t].rearrange("p (h e) -> p h e", h=H)[
                            :, :, 0:OD
                        ],
                        in0=psv[:].rearrange("p (h e) -> p h e", h=H),
                        in1=biasbc[:].rearrange("p (h e) -> p h e", h=H)[
                            :, :, 0:OD
                        ],
                        op=ALU.add,
                    )

                for t in range(KT):
                    vp_tile(t)

                # --- cv: cv1'[h] = sum_k exp(aK_h)[k] * Vp'[k, block h] ---
                # one [8, 264] matmul pair per k-tile; head h's block sits at
                # cols h*66 of the concatenated [8, 528] result (diag blocks)
                with tc.tile_pool(name="cvp", bufs=1, space="PSUM") as cvpp:
                    # [H, 2, 512] so each half's [8, 264] matmul output sits
                    # at a PSUM bank boundary (offsets 0 and 2048 bytes)
                    cvps = cvpp.tile([H, 2, 512], F32, tag="cvps")
                    for t in range(KT):
                        for hh in range(2):
                            nc.tensor.matmul(
                                cvps[:, hh, 0:4 * HB],
                                lhsT=eK1n[:, t],
                                rhs=vp_sb[:, t, hh * 4 * HB:(hh + 1) * 4 * HB],
                                start=(t == 0),
                                stop=(t == KT - 1),
                            )
                    cvf = bigp.tile([H, 2 * 4 * HB], BF16, tag="cvf")
                    for hh in range(2):
                        nc.vector.tensor_copy(
                            out=cvf[:, hh * 4 * HB:(hh + 1) * 4 * HB],
                            in_=cvps[:, hh, 0:4 * HB],
                        )
                # gather diagonal blocks to partition 0: cv_sb[0, h*66+j]
                cv_sb = constp.tile([1, H * HB], BF16, tag="cv")
                for h in range(H):
                    nc.gpsimd.dma_start(
                        out=cv_sb[:, h * HB:(h + 1) * HB],
                        in_=cvf[h:h + 1, h * HB:(h + 1) * HB],
                    )

                # --- Q path (arrives last; staged per half so the grid can
                #     start on q-block 0 as soon as it is projected) ---
                psq = pspairp.tile([2 * H, QS], F32, tag="pair")
                eQ = bigp.tile([2 * H, QS], BF16, tag="eq")
                eqs = bigp.tile([2, H, QS], BF16, tag="eqs")
                for half in range(2):
                    sl = slice(half * 512, (half + 1) * 512)
                    pair_proj(psq, qT, half, wq_sb)
                    nc.scalar.activation(eQ[:, sl], psq[:, sl], AF.Exp)
                    for h in range(H):
                        nc.gpsimd.dma_start(
                            out=eqs[:, h, sl], in_=eQ[2 * h:2 * h + 2, sl]
                        )

            xtcm.__exit__(None, None, None)

            # ---- main grid: score D = B - A, relu, flipped flash ----
            # PSUM accumulation chains within one bank must be strictly
            # sequential (no two open groups in a bank region), so each
            # (qb,h) runs its 4 q-chunk chains back to back; the software
            # pipeline instead runs one full (qb,h) iteration ahead on the
            # score side.
            outv = OUT.rearrange("(t p) e -> p t e", p=128)
            with (
                tc.tile_pool(name="psd", bufs=3, space="PSUM") as psdp,
                tc.tile_pool(name="pso", bufs=2, space="PSUM") as psop,
                tc.tile_pool(name="rpool", bufs=9) as rp,
                tc.tile_pool(name="outf", bufs=4) as outfp,
            ):
                NIT = QB * H  # 16 iterations, j -> (qb, h)
                TP = KT // 2
                psO = [None] * NIT
                rsb = [None] * NIT

                def emit_scores(j):
                    qb, h = divmod(j, H)
                    qs = qb * 512
                    rsb[j] = []
                    for tp in range(TP):
                        ps = psdp.tile([128, 1024], F32, tag="psd",
                                       name=f"psD{j}_{tp}")
                        for i in range(2):
                            t = tp * 2 + i
                            nc.tensor.matmul(
                                ps[:, i * 512:(i + 1) * 512],
                                lhsT=eks[:, h, t * 128:(t + 1) * 128],
                                rhs=eqs[:, h, qs:qs + 512],
                                start=True, stop=True,
                            )
                        r = rp.tile([128, 1024], BF16, tag="r",
                                    name=f"r{j}_{tp}")
                        rsb[j].append(r)
                        # relu split: ACT 576 cols, DVE 448 cols
                        nc.scalar.activation(r[:, 0:576], ps[:, 0:576],
                                             AF.Relu)
                        nc.vector.tensor_scalar(
                            out=r[:, 576:1024], in0=ps[:, 576:1024],
                            scalar1=0.0, scalar2=None, op0=ALU.max,
                        )

                def emit_flashepi(j):
                    qb, h = divmod(j, H)
                    qs = qb * 512
                    pso_t = psop.tile([128, 4 * HB], F32, tag="pso",
                                      name=f"psO{j}")
                    psO[j] = pso_t
                    for c in range(4):
                        # rank-1 A-term opens chunk c's accumulation chain
                        nc.tensor.matmul(
                            pso_t[:, c * HB:(c + 1) * HB],
                            lhsT=eqs[0:1, h, qs + c * 128:qs + (c + 1) * 128],
                            rhs=cv_sb[0:1, h * HB:(h + 1) * HB],
                            start=True, stop=False,
                        )
                        for tp in range(TP):
                            r = rsb[j][tp]
                            for i in range(2):
                                t = tp * 2 + i
                                nc.tensor.matmul(
                                    pso_t[:, c * HB:(c + 1) * HB],
                                    lhsT=r[:, i * 512 + c * 128:
                                           i * 512 + (c + 1) * 128],
                                    rhs=vp_sb[:, t, h * HB:(h + 1) * HB],
                                    start=False, stop=(t == KT - 1),
                                )
                    rsb[j] = None
                    # epilogue: reciprocal of den column, scale, store
                    rden = stagep.tile([128, 4], F32, tag="rden",
                                       name=f"rden{j}")
                    nc.vector.reciprocal(
                        rden[:],
                        pso_t[:].rearrange("p (c e) -> p c e", c=4)[
                            :, :, DEN:DEN + 1
                        ],
                    )
                    oF = outfp.tile([128, 4 * OD], F32, tag="of",
                                    name=f"oF{j}")
                    for c in range(4):
                        nc.vector.tensor_scalar(
                            out=oF[:, c * OD:(c + 1) * OD],
                            in0=pso_t[:, c * HB:c * HB + OD],
                            scalar1=rden[:, c:c + 1],
                            scalar2=None,
                            op0=ALU.mult,
                        )
                    nc.sync.dma_start(
                        out=outv[:, qb * 4:(qb + 1) * 4, h * OD:(h + 1) * OD],
                        in_=oF[:].rearrange("p (c e) -> p c e", c=4),
                    )
                    psO[j] = None

                emit_scores(0)
                for j in range(NIT):
                    if j + 1 < NIT:
                        emit_scores(j + 1)
                    emit_flashepi(j)
    nc.compile()
    return nc


_NC_CACHE = {}


def _get_nc():
    if "nc" not in _NC_CACHE:
        _NC_CACHE["nc"] = build_kernel()
    return _NC_CACHE["nc"]


def make_inmaps(Q, K, V, WQ, WK, WV, bias):
    Q = np.asarray(Q, np.float32)
    K = np.asarray(K, np.float32)
    V = np.asarray(V, np.float32)
    WQ = np.asarray(WQ, np.float32)
    WK = np.asarray(WK, np.float32)
    WV = np.asarray(WV, np.float32)
    bias = np.asarray(bias, np.float32)

    def ext(W):  # (H, D) -> (D, 2H), col 2h = W[h], col 2h+1 = .2*W[h]
        e = np.empty((D, 2 * H), np.float32)
        e[:, 0::2] = W.T
        e[:, 1::2] = NEG * W.T
        return e.astype(ml_dtypes.bfloat16)

    wqe = ext(WQ)
    wke = ext(WK)
    wvt = np.ascontiguousarray(WV.T).astype(ml_dtypes.bfloat16)
    biasext = np.zeros((1, H * HB), np.float32)
    biasext.reshape(H, HB)[:, 0:OD] = bias
    biasext = biasext.astype(ml_dtypes.bfloat16)
    sgn = np.tile(np.array([[-1.0], [1.0]], np.float32), (H, 1))

    # pre-transpose Q/K/V to d-major bf16 (batched transpose, then cast)
    QTb = np.ascontiguousarray(Q.transpose(0, 2, 1)).astype(ml_dtypes.bfloat16)
    KTb = np.ascontiguousarray(K.transpose(0, 2, 1)).astype(ml_dtypes.bfloat16)
    VTb = np.ascontiguousarray(V.transpose(0, 2, 1)).astype(ml_dtypes.bfloat16)

    in_maps = []
    for b in range(NCORES):
        in_maps.append({
            "QT": QTb[b],
            "KT": KTb[b],
            "VT": VTb[b],
            "WQext": wqe,
            "WKext": wke,
            "WVT": wvt,
            "biasext": biasext,
            "sgn": sgn,
        })
    return in_maps


def kernel(Q, K, V, WQ, WK, WV, bias):
    nc = _get_nc()
    in_maps = make_inmaps(Q, K, V, WQ, WK, WV, bias)
    res = run_bass_kernel_spmd(nc, in_maps, list(range(NCORES)))
    out = np.stack([res.results[b]["out"] for b in range(NCORES)], axis=0)
    return out


# revision 11
# speedup vs baseline: 1.3763x; 1.0243x over previous
"""Trainium2 Bass kernel for nn_MhAttnBlock (GAT-style additive attention).

Reference computation (per batch b):
    Vproj = (V @ WV.T).reshape(k, H, 64)
    aK = K @ WK.T   (k, H)
    aQ = Q @ WQ.T   (q, H)
    w  = softmax_k(leaky_relu(aQ[q,h] + aK[k,h], 0.2))
    out[q, h*64+e] = sum_k w[q,k,h] * Vproj[k,h,e] + bias[h,e]

Key algebraic identity used on-device:
    exp(lrelu(s)) for s = aQ+aK equals max(A, B) = A + relu(B - A) with
       A = exp(aQ)*exp(aK)      (rank-1 in (q,k))
       B = exp(.2 aQ)*exp(.2 aK)
    So the score grid needs NO exp: PE builds D = B - A as a contraction-2
    matmul from tiny per-head exp vectors; a relu pass (split across ACT
    and DVE) doubles as the mandatory PSUM->SBUF move; the rank-1 A-term
    folds into the flash matmul as a C=1 accumulation.  Softmax
    denominator = ones column appended to Vproj; bias folds in via
    Vproj += bias (numerator becomes num + bias*den, so num/den = out +
    bias exactly).

This version (v2):
  - All heavy matmuls in bf16 (fp32r's LOW_HIGH replicated mode runs the
    PE at ~half rate / trips the chip power throttle when 8 cores run).
  - Flash matmul flipped: lhsT = relu-grid chunk [128k, 128q], rhs =
    Vproj head block [128k, 66] -> psO [128q, 66].  Output lands q-major
    so the entire PE-transpose epilogue of v1 disappears.
  - Front phase interleaved per-DMA-chunk (K, V, Q load order) so
    transposes/projections hide under the HBM loads.
  - Grid software-pipelined: score matmuls for tile-pair tp+1 issue
    before flash matmuls of tp, so the PE never stalls on the relu.

Sharding: data-parallel over batch B=8 across the 8 NeuronCores.
"""

import sys

for _p in ("/opt/trn_rl_repo", "/root/.axon_site/_ro/trn_rl_repo"):
    if _p not in sys.path:
        sys.path.insert(0, _p)

import numpy as np
import ml_dtypes

import concourse.bass as bass  # noqa: F401
import concourse.bacc as bacc
import concourse.mybir as mybir
import concourse.tile as tile
from concourse.bass_utils import run_bass_kernel_spmd

F32 = mybir.dt.float32
BF16 = mybir.dt.bfloat16
AF = mybir.ActivationFunctionType
ALU = mybir.AluOpType

B, QS, KS = 8, 1024, 1024
D = 512          # qdim = kdim = vdim
H, OD = 8, 64    # heads, head out dim
NEG = 0.2
NCORES = 8

KT = KS // 128   # 8 k-tiles
QT = QS // 128   # 8 q-tiles
DT = D // 128    # 4 d-tiles
QB = QS // 512   # 2 q-blocks of 512
NCH = 4          # dma chunks per input tensor (2 k/q-tiles each)
HB = OD + 2      # 66: [out 64 | den ones | pad]
DEN = OD         # ones/den column index within a head block


def build_kernel():
    nc = bacc.Bacc()

    # host passes Q/K/V pre-transposed (d-major) and bf16-cast; the PE
    # transposes + fp32 loads of earlier versions disappear entirely
    QTd = nc.declare_dram_parameter("QT", [D, QS], BF16, isOutput=False)
    KTd = nc.declare_dram_parameter("KT", [D, KS], BF16, isOutput=False)
    VTd = nc.declare_dram_parameter("VT", [D, KS], BF16, isOutput=False)
    # WQext/WKext: (D, 2H), col 2h = W[h,:], col 2h+1 = 0.2*W[h,:]
    WQe = nc.declare_dram_parameter("WQext", [D, 2 * H], BF16, isOutput=False)
    WKe = nc.declare_dram_parameter("WKext", [D, 2 * H], BF16, isOutput=False)
    WVT = nc.declare_dram_parameter("WVT", [D, D], BF16, isOutput=False)
    # biasext: (1, H*HB): [bias[h,:64], 0, 0] per head
    BIA = nc.declare_dram_parameter("biasext", [1, H * HB], BF16, isOutput=False)
    # sgn: (2H, 1): -1 on even partitions (negates eK1), +1 on odd
    SGN = nc.declare_dram_parameter("sgn", [2 * H, 1], F32, isOutput=False)
    OUT = nc.declare_dram_parameter("out", [QS, H * OD], F32, isOutput=True)

    with tile.TileContext(nc) as tc:
        with (
            tc.tile_pool(name="const", bufs=1) as constp,
            tc.tile_pool(name="big", bufs=1) as bigp,
            tc.tile_pool(name="stage", bufs=3) as stagep,
        ):
            # ---- tiny constants on the gpsimd DGE queue (sync stays free
            #      for the input loads; scalar queue takes the weights) ----
            sgn_sb = constp.tile([2 * H, 1], F32, tag="sgn")
            nc.gpsimd.dma_start(sgn_sb[:], SGN[:])
            biasx = constp.tile([1, H * HB], BF16, tag="biasx")
            nc.gpsimd.dma_start(biasx[:], BIA[:])
            biasbc = constp.tile([128, H * HB], F32, tag="biasbc")
            ones1 = constp.tile([1, 128], BF16, tag="ones1")
            nc.vector.memset(ones1[:], 1.0)
            wk_sb = constp.tile([128, DT, 2 * H], BF16, tag="wk")
            nc.scalar.dma_start(
                wk_sb[:], WKe.rearrange("(dt p) j -> p dt j", p=128)
            )
            wq_sb = constp.tile([128, DT, 2 * H], BF16, tag="wq")
            nc.scalar.dma_start(
                wq_sb[:], WQe.rearrange("(dt p) j -> p dt j", p=128)
            )
            wv_sb = constp.tile([128, DT, D], BF16, tag="wv")
            nc.scalar.dma_start(
                wv_sb[:], WVT.rearrange("(dt p) e -> p dt e", p=128)
            )

            # ---- transposed input loads (K, V, Q order; 4 seq-chunks
            #      each so projections can start as data arrives) ----
            xtcm = tc.tile_pool(name="xt", bufs=1)
            xtp = xtcm.__enter__()

            def load_xt(dram, ns, tag):
                # two s-half chunks: 1KB-per-partition contiguous runs
                t_ = xtp.tile([128, DT, ns], BF16, tag=tag)
                view = dram.rearrange("(dt p) s -> p dt s", p=128)
                for c in range(2):
                    lo, hi = c * ns // 2, (c + 1) * ns // 2
                    nc.sync.dma_start(t_[:, :, lo:hi], view[:, :, lo:hi])
                return t_

            kT = load_xt(KTd, KS, "kT")
            qT = load_xt(QTd, QS, "qT")
            vT = load_xt(VTd, KS, "vT")

            # bias broadcast to 128 partitions via tiny bf16 matmul
            with tc.tile_pool(name="psbb", bufs=1, space="PSUM") as psbbp:
                psbb = psbbp.tile([128, H * HB], F32, tag="psbb")
                nc.tensor.matmul(
                    psbb[:, 0:512], lhsT=ones1[:], rhs=biasx[:, 0:512],
                    start=True, stop=True,
                )
                nc.tensor.matmul(
                    psbb[:, 512:H * HB], lhsT=ones1[:],
                    rhs=biasx[:, 512:H * HB], start=True, stop=True,
                )
                nc.vector.tensor_copy(out=biasbc[:], in_=psbb[:])

            with (
                tc.tile_pool(name="pspair", bufs=1, space="PSUM") as pspairp,
                tc.tile_pool(name="psproj", bufs=2, space="PSUM") as psprojp,
            ):
                # pair-projection psum tiles [2H, seq] fp32 (2 banks each)
                psk = pspairp.tile([2 * H, KS], F32, tag="pair")
                eK = bigp.tile([2 * H, KS], BF16, tag="ek")
                ekf = bigp.tile([2 * H, KS], F32, tag="ekf")
                eK1n = bigp.tile([128, KT, H], BF16, tag="ek1n")

                def pair_proj(ps, xt, half, w_sb):
                    for dt in range(DT):
                        nc.tensor.matmul(
                            ps[:, half * 512:(half + 1) * 512],
                            lhsT=w_sb[:, dt],
                            rhs=xt[:, dt, half * 512:(half + 1) * 512],
                            start=(dt == 0),
                            stop=(dt == DT - 1),
                        )

                def ek_finish(half):
                    sl = slice(half * 512, (half + 1) * 512)
                    nc.scalar.activation(ekf[:, sl], psk[:, sl], AF.Exp)
                    nc.vector.tensor_scalar(
                        out=eK[:, sl], in0=ekf[:, sl], scalar1=sgn_sb[:],
                        scalar2=None, op0=ALU.mult,
                    )

                def ek1n_tile(t):
                    psn_full = psprojp.tile([128, 512], F32, tag="proj",
                                            name=f"psn{t}")
                    psn = psn_full[:, 0:H]
                    for dt in range(DT):
                        nc.tensor.matmul(
                            psn,
                            lhsT=kT[:, dt, t * 128:(t + 1) * 128],
                            rhs=wk_sb[:, dt, 0:2 * H:2],
                            start=(dt == 0),
                            stop=(dt == DT - 1),
                        )
                    nc.scalar.activation(eK1n[:, t], psn, AF.Exp)

                # --- K path ---
                pair_proj(psk, kT, 0, wk_sb)
                ek_finish(0)
                for t in range(0, 4):
                    ek1n_tile(t)
                pair_proj(psk, kT, 1, wk_sb)
                ek_finish(1)
                for t in range(4, 8):
                    ek1n_tile(t)

                # stage eK pair rows at partitions 0/1 (engine APs must
                # start at partition 0/32/64/96; DMA may read anywhere);
                # gpsimd DGE queue so sync stays free for loads
                eks = bigp.tile([2, H, KS], BF16, tag="eks")
                for h in range(H):
                    nc.gpsimd.dma_start(
                        out=eks[:, h], in_=eK[2 * h:2 * h + 2, :]
                    )

                # --- Q path (before V: its projections feed the grid
                #     earliest; staged per half on the now-idle sync queue) ---
                psq = pspairp.tile([2 * H, QS], F32, tag="pair")
                eQ = bigp.tile([2 * H, QS], BF16, tag="eq")
                eqs = bigp.tile([2, H, QS], BF16, tag="eqs")
                for half in range(2):
                    sl = slice(half * 512, (half + 1) * 512)
                    pair_proj(psq, qT, half, wq_sb)
                    nc.scalar.activation(eQ[:, sl], psq[:, sl], AF.Exp)
                    for h in range(H):
                        nc.sync.dma_start(
                            out=eqs[:, h, sl], in_=eQ[2 * h:2 * h + 2, sl]
                        )

                # --- V path: Vproj' per tile: [Vproj_h + bias_h | 1 | 0] ---
                vp_sb = bigp.tile([128, KT, H * HB], BF16, tag="vp")
                nc.vector.memset(
                    vp_sb[:].rearrange("p t (h e) -> p t h e", h=H)[
                        :, :, :, DEN:DEN + 1
                    ],
                    1.0,
                )
                nc.vector.memset(
                    vp_sb[:].rearrange("p t (h e) -> p t h e", h=H)[
                        :, :, :, DEN + 1:HB
                    ],
                    0.0,
                )

                def vp_tile(t):
                    psv = psprojp.tile([128, 512], F32, tag="proj",
                                       name=f"psv{t}")
                    for dt in range(DT):
                        nc.tensor.matmul(
                            psv[:],
                            lhsT=vT[:, dt, t * 128:(t + 1) * 128],
                            rhs=wv_sb[:, dt],
                            start=(dt == 0),
                            stop=(dt == DT - 1),
                        )
                    nc.vector.tensor_tensor(
                        out=vp_sb[:, t].rearrange("p (h e) -> p h e", h=H)[
                            :, :, 0:OD
                        ],
                        in0=psv[:].rearrange("p (h e) -> p h e", h=H),
                        in1=biasbc[:].rearrange("p (h e) -> p h e", h=H)[
                            :, :, 0:OD
                        ],
                        op=ALU.add,
                    )

                for t in range(KT):
                    vp_tile(t)

                # --- cv: cv1'[h] = sum_k exp(aK_h)[k] * Vp'[k, block h] ---
                # one [8, 264] matmul pair per k-tile; head h's block sits at
                # cols h*66 of the concatenated [8, 528] result (diag blocks)
                with tc.tile_pool(name="cvp", bufs=1, space="PSUM") as cvpp:
                    # [H, 2, 512] so each half's [8, 264] matmul output sits
                    # at a PSUM bank boundary (offsets 0 and 2048 bytes)
                    cvps = cvpp.tile([H, 2, 512], F32, tag="cvps")
                    for t in range(KT):
                        for hh in range(2):
                            nc.tensor.matmul(
                                cvps[:, hh, 0:4 * HB],
                                lhsT=eK1n[:, t],
                                rhs=vp_sb[:, t, hh * 4 * HB:(hh + 1) * 4 * HB],
                                start=(t == 0),
                                stop=(t == KT - 1),
                            )
                    cvf = bigp.tile([H, 2 * 4 * HB], BF16, tag="cvf")
                    for hh in range(2):
                        nc.vector.tensor_copy(
                            out=cvf[:, hh * 4 * HB:(hh + 1) * 4 * HB],
                            in_=cvps[:, hh, 0:4 * HB],
                        )
                # gather diagonal blocks to partition 0: cv_sb[0, h*66+j]
                cv_sb = constp.tile([1, H * HB], BF16, tag="cv")
                for h in range(H):
                    nc.gpsimd.dma_start(
                        out=cv_sb[:, h * HB:(h + 1) * HB],
                        in_=cvf[h:h + 1, h * HB:(h + 1) * HB],
                    )

            xtcm.__exit__(None, None, None)

            # ---- main grid: score D = B - A, relu, flipped flash ----
            # PSUM accumulation chains within one bank must be strictly
            # sequential (no two open groups in a bank region), so each
            # (qb,h) runs its 4 q-chunk chains back to back; the software
            # pipeline instead runs one full (qb,h) iteration ahead on the
            # score side.
            outv = OUT.rearrange("(t p) e -> p t e", p=128)
            with (
                tc.tile_pool(name="psd", bufs=3, space="PSUM") as psdp,
                tc.tile_pool(name="pso", bufs=2, space="PSUM") as psop,
                tc.tile_pool(name="rpool", bufs=9) as rp,
                tc.tile_pool(name="outf", bufs=4) as outfp,
            ):
                NIT = QB * H  # 16 iterations, j -> (qb, h)
                TP = KT // 2
                psO = [None] * NIT
                rsb = [None] * NIT

                def emit_scores(j):
                    qb, h = divmod(j, H)
                    qs = qb * 512
                    rsb[j] = []
                    for tp in range(TP):
                        ps = psdp.tile([128, 1024], F32, tag="psd",
                                       name=f"psD{j}_{tp}")
                        for i in range(2):
                            t = tp * 2 + i
                            nc.tensor.matmul(
                                ps[:, i * 512:(i + 1) * 512],
                                lhsT=eks[:, h, t * 128:(t + 1) * 128],
                                rhs=eqs[:, h, qs:qs + 512],
                                start=True, stop=True,
                            )
                        r = rp.tile([128, 1024], BF16, tag="r",
                                    name=f"r{j}_{tp}")
                        rsb[j].append(r)
                        # relu split: ACT 576 cols, DVE 448 cols
                        nc.scalar.activation(r[:, 0:576], ps[:, 0:576],
                                             AF.Relu)
                        nc.vector.tensor_scalar(
                            out=r[:, 576:1024], in0=ps[:, 576:1024],
                            scalar1=0.0, scalar2=None, op0=ALU.max,
                        )

                def emit_flashepi(j):
                    qb, h = divmod(j, H)
                    qs = qb * 512
                    pso_t = psop.tile([128, 4 * HB], F32, tag="pso",
                                      name=f"psO{j}")
                    psO[j] = pso_t
                    for c in range(4):
                        # rank-1 A-term opens chunk c's accumulation chain
                        nc.tensor.matmul(
                            pso_t[:, c * HB:(c + 1) * HB],
                            lhsT=eqs[0:1, h, qs + c * 128:qs + (c + 1) * 128],
                            rhs=cv_sb[0:1, h * HB:(h + 1) * HB],
                            start=True, stop=False,
                        )
                        for tp in range(TP):
                            r = rsb[j][tp]
                            for i in range(2):
                                t = tp * 2 + i
                                nc.tensor.matmul(
                                    pso_t[:, c * HB:(c + 1) * HB],
                                    lhsT=r[:, i * 512 + c * 128:
                                           i * 512 + (c + 1) * 128],
                                    rhs=vp_sb[:, t, h * HB:(h + 1) * HB],
                                    start=False, stop=(t == KT - 1),
                                )
                    rsb[j] = None
                    # epilogue: reciprocal of den column, scale, store
                    rden = stagep.tile([128, 4], F32, tag="rden",
                                       name=f"rden{j}")
                    nc.vector.reciprocal(
                        rden[:],
                        pso_t[:].rearrange("p (c e) -> p c e", c=4)[
                            :, :, DEN:DEN + 1
                        ],
                    )
                    oF = outfp.tile([128, 4 * OD], F32, tag="of",
                                    name=f"oF{j}")
                    for c in range(4):
                        nc.vector.tensor_scalar(
                            out=oF[:, c * OD:(c + 1) * OD],
                            in0=pso_t[:, c * HB:c * HB + OD],
                            scalar1=rden[:, c:c + 1],
                            scalar2=None,
                            op0=ALU.mult,
                        )
                    nc.sync.dma_start(
                        out=outv[:, qb * 4:(qb + 1) * 4, h * OD:(h + 1) * OD],
                        in_=oF[:].rearrange("p (c e) -> p c e", c=4),
                    )
                    psO[j] = None

                emit_scores(0)
                for j in range(NIT):
                    if j + 1 < NIT:
                        emit_scores(j + 1)
                    emit_flashepi(j)
    nc.compile()
    return nc


_NC_CACHE = {}


def _get_nc():
    if "nc" not in _NC_CACHE:
        _NC_CACHE["nc"] = build_kernel()
    return _NC_CACHE["nc"]


def make_inmaps(Q, K, V, WQ, WK, WV, bias):
    Q = np.asarray(Q, np.float32)
    K = np.asarray(K, np.float32)
    V = np.asarray(V, np.float32)
    WQ = np.asarray(WQ, np.float32)
    WK = np.asarray(WK, np.float32)
    WV = np.asarray(WV, np.float32)
    bias = np.asarray(bias, np.float32)

    def ext(W):  # (H, D) -> (D, 2H), col 2h = W[h], col 2h+1 = .2*W[h]
        e = np.empty((D, 2 * H), np.float32)
        e[:, 0::2] = W.T
        e[:, 1::2] = NEG * W.T
        return e.astype(ml_dtypes.bfloat16)

    wqe = ext(WQ)
    wke = ext(WK)
    wvt = np.ascontiguousarray(WV.T).astype(ml_dtypes.bfloat16)
    biasext = np.zeros((1, H * HB), np.float32)
    biasext.reshape(H, HB)[:, 0:OD] = bias
    biasext = biasext.astype(ml_dtypes.bfloat16)
    sgn = np.tile(np.array([[-1.0], [1.0]], np.float32), (H, 1))

    # pre-transpose Q/K/V to d-major bf16 (batched transpose, then cast)
    QTb = np.ascontiguousarray(Q.transpose(0, 2, 1)).astype(ml_dtypes.bfloat16)
    KTb = np.ascontiguousarray(K.transpose(0, 2, 1)).astype(ml_dtypes.bfloat16)
    VTb = np.ascontiguousarray(V.transpose(0, 2, 1)).astype(ml_dtypes.bfloat16)

    in_maps = []
    for b in range(NCORES):
        in_maps.append({
            "QT": QTb[b],
            "KT": KTb[b],
            "VT": VTb[b],
            "WQext": wqe,
            "WKext": wke,
            "WVT": wvt,
            "biasext": biasext,
            "sgn": sgn,
        })
    return in_maps


def kernel(Q, K, V, WQ, WK, WV, bias):
    nc = _get_nc()
    in_maps = make_inmaps(Q, K, V, WQ, WK, WV, bias)
    res = run_bass_kernel_spmd(nc, in_maps, list(range(NCORES)))
    out = np.stack([res.results[b]["out"] for b in range(NCORES)], axis=0)
    return out
